# revision 7
# baseline (speedup 1.0000x reference)
# Trainium2 Bass kernel for nn_Model_26190710571339 (topk_masking).
#
# Model: scores = einsum('bnf,f->bn', feats, w_conv); per-bag sort -> bottom-5
# and top-5 score values -> tiny MLP (10->200->100->1, sigmoid) -> logits, probs.
#
# Sharding: data-parallel over the bag axis; 2 bags per NeuronCore x 8 cores.
# Weights replicated.
#
# v3: - feats staged as fp8_e4m3, host-transposed (f on partitions); scores
#       computed on the TensorEngine: per 128-tile block, 16 accumulating
#       matmuls (lhsT = feats^T chunk [128f x 128n] fp8, rhs = w chunk
#       [128f x 1] fp16) put the 128 scores across PSUM partitions. 4x fewer
#       DMA bytes than f32 (67 MB/core); PE (~120 us) hides under the DMA
#       stream (~180 us). End-to-end quantization error vs the f32 reference
#       (numpy sim on the actual inputs): rel ~3.4e-3 on logits (gate 2e-2).
#     - 64 x 1 MB windows so the PE's DMA-completion dependency is fine-
#       grained (the 3 DMA queues interleave at the engine level, so a
#       window completes ~3 window-times after issue).
#     - top/bottom-5 via the DVE max8 instruction: top-8 per partition
#       (exact: one partition contributes at most 5 of a bag's top-5),
#       gather to one row, one final max8. Bottom side runs max8 on
#       negated scores; the sign flip and the descending value order are
#       folded into a host-side permutation of W1's columns. Bag 0's
#       candidate phase overlaps the bag-1 half of the stream.

import numpy as np

B = 16
NTILES = 16384
FSZ = 2048
R = 5
NCORES = 8
BAGS_PER_CORE = B // NCORES  # 2

NWIN = 64            # DMA/compute windows per core
WINN = 512           # tiles (scores) per window
NCHUNK = FSZ // 128  # 16 f-chunks of 128


def _build_nc(nbags, ntiles, fsz, bufs=12, ncores=NCORES):
    import concourse.mybir as mybir
    import concourse.tile as tile
    from concourse import bacc
    from contextlib import ExitStack

    f32 = mybir.dt.float32
    f16 = mybir.dt.float16
    f8 = mybir.dt.float8e4
    Act = mybir.ActivationFunctionType

    rows = nbags * ntiles
    nblk = rows // 128            # number of score columns (256)
    cols_per_bag = ntiles // 128  # 128
    assert nblk == NWIN * (WINN // 128)
    win_per_bag = NWIN // nbags

    nc = bacc.Bacc("TRN2", target_bir_lowering=False, debug=False, num_devices=ncores)
    ft8 = nc.declare_dram_parameter("ft8", [NWIN, 128, NCHUNK * WINN], f8, isOutput=False)
    w16 = nc.declare_dram_parameter("w16", [128, NCHUNK], f16, isOutput=False)
    w1t = nc.declare_dram_parameter("w1t", [2 * R, 200], f32, isOutput=False)
    w2ta = nc.declare_dram_parameter("w2ta", [128, 100], f32, isOutput=False)
    w2tb = nc.declare_dram_parameter("w2tb", [72, 100], f32, isOutput=False)
    w3t = nc.declare_dram_parameter("w3t", [100, 1], f32, isOutput=False)
    b1a = nc.declare_dram_parameter("b1a", [128, 1], f32, isOutput=False)
    b1b = nc.declare_dram_parameter("b1b", [72, 1], f32, isOutput=False)
    b2c = nc.declare_dram_parameter("b2c", [100, 1], f32, isOutput=False)
    b3c = nc.declare_dram_parameter("b3c", [1, 1], f32, isOutput=False)
    idn = nc.declare_dram_parameter("idn", [nbags, nbags], f32, isOutput=False)
    logits_o = nc.declare_dram_parameter("logits", [1, nbags], f32, isOutput=True)
    probs_o = nc.declare_dram_parameter("probs", [1, nbags], f32, isOutput=True)

    with ExitStack() as ctx:
        tc = ctx.enter_context(tile.TileContext(nc))
        consts = ctx.enter_context(tc.tile_pool(name="consts", bufs=1))

        # w16 is needed by the very first matmul: issue its DMA first.
        w16_sb = consts.tile([128, NCHUNK], f16)
        nc.sync.dma_start(w16_sb[:], w16[:])

        scores = consts.tile([128, nblk], f32)

        # ---- main loop: stream fp8 transposed windows, PE matmul -> scores
        fpool = ctx.enter_context(tc.tile_pool(name="fpool", bufs=bufs))
        psum = ctx.enter_context(tc.tile_pool(name="psum", bufs=3, space="PSUM"))
        tpool = ctx.enter_context(tc.tile_pool(name="tpool", bufs=1))
        dma_rings = [nc.sync, nc.scalar, nc.gpsimd]
        nb_per_win = WINN // 128  # 4

        # per-bag candidate tiles
        negsc = [tpool.tile([128, cols_per_bag], f32, name=f"negsc{b}") for b in range(nbags)]
        cmax8 = [tpool.tile([128, 8], f32, name=f"cmax8{b}") for b in range(nbags)]
        cmin8 = [tpool.tile([128, 8], f32, name=f"cmin8{b}") for b in range(nbags)]
        cand_max = tpool.tile([nbags, 128 * 8], f32)
        cand_min = tpool.tile([nbags, 128 * 8], f32)

        def bag_candidates(b):
            # DVE-only (in-order on DVE, so this never stalls the DMA
            # issuers); the cross-partition gather DMAs are issued post-loop
            # so they don't block later window issues on their ring.
            sc_b = scores[:, b * cols_per_bag : (b + 1) * cols_per_bag]
            nc.vector.tensor_scalar_mul(negsc[b][:], sc_b, -1.0)
            nc.vector.max(cmax8[b][:], sc_b)
            nc.vector.max(cmin8[b][:], negsc[b][:])

        for w in range(NWIN):
            ftw = fpool.tile([128, NCHUNK * WINN], f8, name="ftw")
            dma_rings[w % len(dma_rings)].dma_start(ftw[:], ft8[w])
            pt = psum.tile([128, nb_per_win], f32, name="pt")
            for b in range(nb_per_win):
                for c in range(NCHUNK):
                    nc.tensor.matmul(
                        pt[:, b : b + 1],
                        lhsT=ftw[:, c * WINN + b * 128 : c * WINN + (b + 1) * 128],
                        rhs=w16_sb[:, c : c + 1],
                        start=(c == 0),
                        stop=(c == NCHUNK - 1),
                    )
            nc.vector.tensor_copy(scores[:, w * nb_per_win : (w + 1) * nb_per_win], pt[:])
            # bag b's scores are complete after its last window: kick off its
            # candidate reduction so only bag nbags-1's runs in the tail.
            if (w + 1) % win_per_bag == 0:
                bag_candidates((w + 1) // win_per_bag - 1)

        for b in range(nbags):
            nc.sync.dma_start(cand_max[b : b + 1, :], cmax8[b][:])
            nc.scalar.dma_start(cand_min[b : b + 1, :], cmin8[b][:])

        # MLP consts: only needed in the tail; issue after the stream DMAs.
        w1t_sb = consts.tile([2 * R, 200], f32)
        nc.scalar.dma_start(w1t_sb[:], w1t[:])
        w2ta_sb = consts.tile([128, 100], f32)
        nc.scalar.dma_start(w2ta_sb[:], w2ta[:])
        w2tb_sb = consts.tile([72, 100], f32)
        nc.scalar.dma_start(w2tb_sb[:], w2tb[:])
        w3t_sb = consts.tile([100, 1], f32)
        nc.scalar.dma_start(w3t_sb[:], w3t[:])
        b1a_sb = consts.tile([128, 1], f32)
        nc.scalar.dma_start(b1a_sb[:], b1a[:])
        b1b_sb = consts.tile([72, 1], f32)
        nc.scalar.dma_start(b1b_sb[:], b1b[:])
        b2c_sb = consts.tile([100, 1], f32)
        nc.scalar.dma_start(b2c_sb[:], b2c[:])
        b3c_sb = consts.tile([1, 1], f32)
        nc.scalar.dma_start(b3c_sb[:], b3c[:])
        idn_sb = consts.tile([nbags, nbags], f32)
        nc.scalar.dma_start(idn_sb[:], idn[:])

        # ---- global top/bottom 8 across each bag's 1024 candidates.
        # mm layout (host-permuted W1 compensates): mm[b, 0:5] = bottom-5
        # negated+ascending-by-magnitude = cmin8 desc order; mm[b, 5:10] =
        # top-5 descending = cmax8 desc order.
        g8max = tpool.tile([nbags, 8], f32)
        g8min = tpool.tile([nbags, 8], f32)
        nc.vector.max(g8max[:], cand_max[:])
        nc.vector.max(g8min[:], cand_min[:])
        minmax = tpool.tile([nbags, 2 * R], f32)
        nc.vector.tensor_copy(minmax[:, 0:R], g8min[:, 0:R])
        nc.vector.tensor_copy(minmax[:, R : 2 * R], g8max[:, 0:R])

        # ---- MLP (transposed): hT = sigmoid(W @ xT + b), biases per-partition
        psum2 = ctx.enter_context(tc.tile_pool(name="psum2", bufs=1, space="PSUM"))
        mmT_ps = psum2.tile([2 * R, nbags], f32, name="mmT_ps")
        nc.tensor.transpose(mmT_ps[:], minmax[:], idn_sb[:])
        mmT = tpool.tile([2 * R, nbags], f32)
        nc.vector.tensor_copy(mmT[:], mmT_ps[:])

        h1pa = psum2.tile([128, nbags], f32, name="h1pa")
        h1pb = psum2.tile([72, nbags], f32, name="h1pb")
        nc.tensor.matmul(h1pa[:], lhsT=w1t_sb[:, 0:128], rhs=mmT[:], start=True, stop=True)
        nc.tensor.matmul(h1pb[:], lhsT=w1t_sb[:, 128:200], rhs=mmT[:], start=True, stop=True)
        h1a = tpool.tile([128, nbags], f32)
        h1b = tpool.tile([72, nbags], f32)
        nc.scalar.activation(h1a[:], h1pa[:], Act.Sigmoid, bias=b1a_sb[:], scale=1.0)
        nc.scalar.activation(h1b[:], h1pb[:], Act.Sigmoid, bias=b1b_sb[:], scale=1.0)

        h2p = psum2.tile([100, nbags], f32, name="h2p")
        nc.tensor.matmul(h2p[:], lhsT=w2ta_sb[:], rhs=h1a[:], start=True, stop=False)
        nc.tensor.matmul(h2p[:], lhsT=w2tb_sb[:], rhs=h1b[:], start=False, stop=True)
        h2 = tpool.tile([100, nbags], f32)
        nc.scalar.activation(h2[:], h2p[:], Act.Sigmoid, bias=b2c_sb[:], scale=1.0)

        lp = psum2.tile([1, nbags], f32, name="lp")
        nc.tensor.matmul(lp[:], lhsT=w3t_sb[:], rhs=h2[:], start=True, stop=True)
        lsb = tpool.tile([1, nbags], f32)
        nc.vector.tensor_scalar_add(lsb[:], lp[:], b3c_sb[:])
        psb = tpool.tile([1, nbags], f32)
        nc.scalar.activation(psb[:], lsb[:], Act.Sigmoid)

        nc.sync.dma_start(logits_o[:], lsb[:])
        nc.sync.dma_start(probs_o[:], psb[:])

    nc.finalize()
    return nc


def _make_in_maps(inputs, nbags, ntiles, fsz, ncores):
    import ml_dtypes

    feats = np.asarray(inputs["feats"], dtype=np.float32)
    w_conv = np.asarray(inputs["w_conv"], dtype=np.float32)
    W1 = np.asarray(inputs["W1"], dtype=np.float32)
    b1 = np.asarray(inputs["b1"], dtype=np.float32)
    W2 = np.asarray(inputs["W2"], dtype=np.float32)
    b2 = np.asarray(inputs["b2"], dtype=np.float32)
    W3 = np.asarray(inputs["W3"], dtype=np.float32)
    b3 = np.asarray(inputs["b3"], dtype=np.float32)

    # Kernel produces mm[b, 0:5] = -(bottom-5 ascending) and
    # mm[b, 5:10] = top-5 descending; reference minmax is bottom-5 ascending
    # then top-5 ascending. Fold both differences into W1's columns.
    W1_hw = np.empty_like(W1)
    W1_hw[:, 0:R] = -W1[:, 0:R]
    W1_hw[:, R : 2 * R] = W1[:, 2 * R - 1 : R - 1 : -1]

    base = {
        # w16[p, c] = w_conv[c*128 + p]
        "w16": np.ascontiguousarray(w_conv.reshape(NCHUNK, 128).T.astype(np.float16)),
        "w1t": np.ascontiguousarray(W1_hw.T),
        "w2ta": np.ascontiguousarray(W2.T[:128]),
        "w2tb": np.ascontiguousarray(W2.T[128:]),
        "w3t": np.ascontiguousarray(W3.T),
        "b1a": np.ascontiguousarray(b1[:128].reshape(128, 1)),
        "b1b": np.ascontiguousarray(b1[128:].reshape(72, 1)),
        "b2c": np.ascontiguousarray(b2.reshape(100, 1)),
        "b3c": np.ascontiguousarray(b3.reshape(1, 1)),
        "idn": np.eye(nbags, dtype=np.float32),
    }
    in_maps = []
    for cid in range(ncores):
        shard = feats[cid * nbags : (cid + 1) * nbags].reshape(nbags * ntiles, fsz)
        q = shard.astype(ml_dtypes.float8_e4m3)
        # A[w, p, c, n_in] = q[w*WINN + n_in, c*128 + p]
        a = q.reshape(NWIN, WINN, NCHUNK, 128).transpose(0, 3, 2, 1)
        a = np.ascontiguousarray(a).reshape(NWIN, 128, NCHUNK * WINN)
        in_maps.append({**base, "ft8": a})
    return in_maps


def _run(inputs, trace=False, **spmd_kwargs):
    from concourse.bass_utils import run_bass_kernel_spmd

    nc = _build_nc(BAGS_PER_CORE, NTILES, FSZ)
    in_maps = _make_in_maps(inputs, BAGS_PER_CORE, NTILES, FSZ, NCORES)
    res = run_bass_kernel_spmd(
        nc, in_maps, list(range(NCORES)), trace=trace, **spmd_kwargs
    )
    logits = np.concatenate(
        [res.results[c]["logits"].reshape(BAGS_PER_CORE, 1) for c in range(NCORES)],
        axis=0,
    )
    probs = np.concatenate(
        [res.results[c]["probs"].reshape(BAGS_PER_CORE, 1) for c in range(NCORES)],
        axis=0,
    )
    return (logits, probs), res


def kernel(**inputs):
    out, _ = _run(inputs, trace=False)
    return out


# revision 11
# speedup vs baseline: 1.0984x; 1.0984x over previous
# Trainium2 Bass kernel for nn_Model_26190710571339 (topk_masking).
#
# Model: scores = einsum('bnf,f->bn', feats, w_conv); per-bag sort -> bottom-5
# and top-5 score values -> tiny MLP (10->200->100->1, sigmoid) -> logits, probs.
#
# Sharding: data-parallel over the bag axis; 2 bags per NeuronCore x 8 cores.
# Weights replicated.
#
# v4: - feats staged as fp8_e4m3, host-transposed (f on partitions); scores
#       are computed on the TensorEngine and kept RESIDENT IN PSUM
#       ([128, 256] f32, half a bank) while f-chunk stripes stream in:
#       stripe (h, c) = [128 partitions = f-chunk c, 16384 tiles of bag-half
#       h], fully contiguous 16 KB per partition -> maximum DMA engine
#       efficiency (~27 GB/s/engine at 16-32 KB descriptors vs ~21 at 8 KB).
#       Each stripe feeds 128 single-column accumulating matmuls
#       (lhsT = stripe[:, b*128:(b+1)*128] fp8, rhs = w chunk [128, 1] fp16).
#       fp8 cuts DMA bytes 4x vs f32 (67 MB/core); PE (~120 us total) hides
#       under the ~165 us DMA stream. Quantization rel err ~3.4e-3 (gate 2e-2).
#     - first and last stripes are issued as 4 KB-line quarters so the PE
#       ramps as soon as ~1 MB has landed and drains ~1 us after the stream.
#     - stream on the two HWDGE queues (SP+ACT) only; gpsimd's SWDGE queue
#       stays empty for the small late DMAs (candidate gathers, MLP consts,
#       outputs) so they never wait behind stream traffic.
#     - top/bottom-5 via DVE max8: top-8 per partition (exact: one partition
#       row holds at most 5 of a bag's global top-5), gather to one row per
#       bag, one final max8. Bottom side runs max8 on negated scores; the
#       sign flip and descending order are folded into a host-side
#       permutation of W1's columns. Bag 0's phase overlaps the stream.

import numpy as np

B = 16
NTILES = 16384
FSZ = 2048
R = 5
NCORES = 8
BAGS_PER_CORE = B // NCORES  # 2

NCHUNK = FSZ // 128   # 16 f-chunks of 128
NSTRIPE = 2 * NCHUNK  # 32 stripes: s = h*16 + c
SLEN = 16384          # tiles per stripe (one bag-half)
NQ = 4                # quarters for the first/last stripe


def _build_nc(nbags, ntiles, fsz, bufs=9, ncores=NCORES):
    import concourse.mybir as mybir
    import concourse.tile as tile
    from concourse import bacc
    from contextlib import ExitStack

    f32 = mybir.dt.float32
    f16 = mybir.dt.float16
    f8 = mybir.dt.float8e4
    Act = mybir.ActivationFunctionType

    rows = nbags * ntiles
    nblk = rows // 128            # number of score columns (256)
    cols_per_bag = ntiles // 128  # 128

    nc = bacc.Bacc("TRN2", target_bir_lowering=False, debug=False, num_devices=ncores)
    ft8 = nc.declare_dram_parameter("ft8", [NSTRIPE, 128, SLEN], f8, isOutput=False)
    w16 = nc.declare_dram_parameter("w16", [128, NCHUNK], f16, isOutput=False)
    w1t = nc.declare_dram_parameter("w1t", [2 * R, 200], f32, isOutput=False)
    w2ta = nc.declare_dram_parameter("w2ta", [128, 100], f32, isOutput=False)
    w2tb = nc.declare_dram_parameter("w2tb", [72, 100], f32, isOutput=False)
    w3t = nc.declare_dram_parameter("w3t", [100, 1], f32, isOutput=False)
    b1a = nc.declare_dram_parameter("b1a", [128, 1], f32, isOutput=False)
    b1b = nc.declare_dram_parameter("b1b", [72, 1], f32, isOutput=False)
    b2c = nc.declare_dram_parameter("b2c", [100, 1], f32, isOutput=False)
    b3c = nc.declare_dram_parameter("b3c", [1, 1], f32, isOutput=False)
    idn = nc.declare_dram_parameter("idn", [nbags, nbags], f32, isOutput=False)
    logits_o = nc.declare_dram_parameter("logits", [1, nbags], f32, isOutput=True)
    probs_o = nc.declare_dram_parameter("probs", [1, nbags], f32, isOutput=True)

    with ExitStack() as ctx:
        tc = ctx.enter_context(tile.TileContext(nc))
        consts = ctx.enter_context(tc.tile_pool(name="consts", bufs=1))

        # w16 is needed by the very first matmul: issue its DMA first.
        w16_sb = consts.tile([128, NCHUNK], f16)
        nc.sync.dma_start(w16_sb[:], w16[:])

        scores = consts.tile([128, nblk], f32)

        # ---- main loop: stream fp8 transposed stripes, PE matmul -> PSUM
        fpool = ctx.enter_context(tc.tile_pool(name="fpool", bufs=bufs))
        qpool = ctx.enter_context(tc.tile_pool(name="qpool", bufs=NQ))
        psmain = ctx.enter_context(tc.tile_pool(name="psmain", bufs=1, space="PSUM"))
        tpool = ctx.enter_context(tc.tile_pool(name="tpool", bufs=1))
        hwdge = [nc.sync, nc.scalar]
        pscore = psmain.tile([128, nblk], f32)

        # Zero pscore and its has_written bits with one all-zero matmul so
        # the per-column accumulation below can use start=False throughout:
        # a start=True matmul clears has_written for the whole PSUM bank,
        # which would wipe other columns' partial sums mid-stream.
        zt = consts.tile([128, nblk], f8)
        nc.vector.memset(zt[:], 0)
        nc.tensor.matmul(
            pscore[:], lhsT=zt[:, 0:128], rhs=zt[:], start=True, stop=True
        )

        negsc = [tpool.tile([128, cols_per_bag], f32, name=f"negsc{b}") for b in range(nbags)]
        cmax8 = [tpool.tile([128, 8], f32, name=f"cmax8{b}") for b in range(nbags)]
        cmin8 = [tpool.tile([128, 8], f32, name=f"cmin8{b}") for b in range(nbags)]
        cand_max = tpool.tile([nbags, 128 * 8], f32)
        cand_min = tpool.tile([nbags, 128 * 8], f32)

        ndma = 0

        def stripe_mms(tile_ap, h, c, b0, nb):
            # tile_ap holds stripe tiles [128, nb*128] for nblocks b0..b0+nb
            for b in range(nb):
                col = h * cols_per_bag + b0 + b
                nc.tensor.matmul(
                    pscore[:, col : col + 1],
                    lhsT=tile_ap[:, b * 128 : (b + 1) * 128],
                    rhs=w16_sb[:, c : c + 1],
                    start=False,
                    stop=(c == NCHUNK - 1),
                    skip_group_check=True,
                )

        for s in range(NSTRIPE):
            h, c = divmod(s, NCHUNK)
            if s == 0 or s == NSTRIPE - 1:
                # quarter-stripes: fast PE ramp at the head, fast drain at
                # the tail (4 KB descriptors, only ~3% of total bytes)
                ql = SLEN // NQ
                for q in range(NQ):
                    qt = qpool.tile([128, ql], f8, name="qt")
                    hwdge[ndma % 2].dma_start(qt[:], ft8[s][:, q * ql : (q + 1) * ql])
                    ndma += 1
                    stripe_mms(qt[:], h, c, q * (ql // 128), ql // 128)
            else:
                st = fpool.tile([128, SLEN], f8, name="st")
                hwdge[ndma % 2].dma_start(st[:], ft8[s])
                ndma += 1
                stripe_mms(st[:], h, c, 0, SLEN // 128)
            if s == NCHUNK - 1 or s == NSTRIPE - 1:
                # bag-half h complete: copy its scores out of PSUM and run
                # the per-partition candidate reduction (DVE only; gathers
                # are issued on the idle SWDGE queue below).
                bsl = slice(h * cols_per_bag, (h + 1) * cols_per_bag)
                nc.vector.tensor_copy(scores[:, bsl], pscore[:, bsl])
                sc_b = scores[:, bsl]
                nc.vector.tensor_scalar_mul(negsc[h][:], sc_b, -1.0)
                nc.vector.max(cmax8[h][:], sc_b)
                nc.vector.max(cmin8[h][:], negsc[h][:])

        # small DMAs on the empty SWDGE queue: consts immediately, candidate
        # gathers as soon as their semaphores fire.
        w1t_sb = consts.tile([2 * R, 200], f32)
        nc.gpsimd.dma_start(w1t_sb[:], w1t[:])
        w2ta_sb = consts.tile([128, 100], f32)
        nc.gpsimd.dma_start(w2ta_sb[:], w2ta[:])
        w2tb_sb = consts.tile([72, 100], f32)
        nc.gpsimd.dma_start(w2tb_sb[:], w2tb[:])
        w3t_sb = consts.tile([100, 1], f32)
        nc.gpsimd.dma_start(w3t_sb[:], w3t[:])
        b1a_sb = consts.tile([128, 1], f32)
        nc.gpsimd.dma_start(b1a_sb[:], b1a[:])
        b1b_sb = consts.tile([72, 1], f32)
        nc.gpsimd.dma_start(b1b_sb[:], b1b[:])
        b2c_sb = consts.tile([100, 1], f32)
        nc.gpsimd.dma_start(b2c_sb[:], b2c[:])
        b3c_sb = consts.tile([1, 1], f32)
        nc.gpsimd.dma_start(b3c_sb[:], b3c[:])
        idn_sb = consts.tile([nbags, nbags], f32)
        nc.gpsimd.dma_start(idn_sb[:], idn[:])
        for b in range(nbags):
            nc.gpsimd.dma_start(cand_max[b : b + 1, :], cmax8[b][:])
            nc.gpsimd.dma_start(cand_min[b : b + 1, :], cmin8[b][:])

        # ---- global top/bottom 8 across each bag's 1024 candidates.
        g8max = tpool.tile([nbags, 8], f32)
        g8min = tpool.tile([nbags, 8], f32)
        nc.vector.max(g8max[:], cand_max[:])
        nc.vector.max(g8min[:], cand_min[:])
        # mm[b, 0:5] = bottom-5 negated (descending |.|), mm[b, 5:10] =
        # top-5 descending; host-permuted W1 compensates.
        minmax = tpool.tile([nbags, 2 * R], f32)
        nc.vector.tensor_copy(minmax[:, 0:R], g8min[:, 0:R])
        nc.vector.tensor_copy(minmax[:, R : 2 * R], g8max[:, 0:R])

        # ---- MLP (transposed): hT = sigmoid(W @ xT + b), biases per-partition
        psum2 = ctx.enter_context(tc.tile_pool(name="psum2", bufs=1, space="PSUM"))
        mmT_ps = psum2.tile([2 * R, nbags], f32, name="mmT_ps")
        nc.tensor.transpose(mmT_ps[:], minmax[:], idn_sb[:])
        mmT = tpool.tile([2 * R, nbags], f32)
        nc.vector.tensor_copy(mmT[:], mmT_ps[:])

        h1pa = psum2.tile([128, nbags], f32, name="h1pa")
        h1pb = psum2.tile([72, nbags], f32, name="h1pb")
        nc.tensor.matmul(h1pa[:], lhsT=w1t_sb[:, 0:128], rhs=mmT[:], start=True, stop=True)
        nc.tensor.matmul(h1pb[:], lhsT=w1t_sb[:, 128:200], rhs=mmT[:], start=True, stop=True)
        h1a = tpool.tile([128, nbags], f32)
        h1b = tpool.tile([72, nbags], f32)
        nc.scalar.activation(h1a[:], h1pa[:], Act.Sigmoid, bias=b1a_sb[:], scale=1.0)
        nc.scalar.activation(h1b[:], h1pb[:], Act.Sigmoid, bias=b1b_sb[:], scale=1.0)

        h2p = psum2.tile([100, nbags], f32, name="h2p")
        nc.tensor.matmul(h2p[:], lhsT=w2ta_sb[:], rhs=h1a[:], start=True, stop=False)
        nc.tensor.matmul(h2p[:], lhsT=w2tb_sb[:], rhs=h1b[:], start=False, stop=True)
        h2 = tpool.tile([100, nbags], f32)
        nc.scalar.activation(h2[:], h2p[:], Act.Sigmoid, bias=b2c_sb[:], scale=1.0)

        lp = psum2.tile([1, nbags], f32, name="lp")
        nc.tensor.matmul(lp[:], lhsT=w3t_sb[:], rhs=h2[:], start=True, stop=True)
        lsb = tpool.tile([1, nbags], f32)
        nc.vector.tensor_scalar_add(lsb[:], lp[:], b3c_sb[:])
        psb = tpool.tile([1, nbags], f32)
        nc.scalar.activation(psb[:], lsb[:], Act.Sigmoid)

        nc.gpsimd.dma_start(logits_o[:], lsb[:])
        nc.gpsimd.dma_start(probs_o[:], psb[:])

    nc.finalize()
    return nc


def _make_in_maps(inputs, nbags, ntiles, fsz, ncores):
    import ml_dtypes

    feats = np.asarray(inputs["feats"], dtype=np.float32)
    w_conv = np.asarray(inputs["w_conv"], dtype=np.float32)
    W1 = np.asarray(inputs["W1"], dtype=np.float32)
    b1 = np.asarray(inputs["b1"], dtype=np.float32)
    W2 = np.asarray(inputs["W2"], dtype=np.float32)
    b2 = np.asarray(inputs["b2"], dtype=np.float32)
    W3 = np.asarray(inputs["W3"], dtype=np.float32)
    b3 = np.asarray(inputs["b3"], dtype=np.float32)

    # Kernel produces mm[b, 0:5] = -(bottom-5 ascending) and
    # mm[b, 5:10] = top-5 descending; reference minmax is bottom-5 ascending
    # then top-5 ascending. Fold both differences into W1's columns.
    W1_hw = np.empty_like(W1)
    W1_hw[:, 0:R] = -W1[:, 0:R]
    W1_hw[:, R : 2 * R] = W1[:, 2 * R - 1 : R - 1 : -1]

    base = {
        # w16[p, c] = w_conv[c*128 + p]
        "w16": np.ascontiguousarray(w_conv.reshape(NCHUNK, 128).T.astype(np.float16)),
        "w1t": np.ascontiguousarray(W1_hw.T),
        "w2ta": np.ascontiguousarray(W2.T[:128]),
        "w2tb": np.ascontiguousarray(W2.T[128:]),
        "w3t": np.ascontiguousarray(W3.T),
        "b1a": np.ascontiguousarray(b1[:128].reshape(128, 1)),
        "b1b": np.ascontiguousarray(b1[128:].reshape(72, 1)),
        "b2c": np.ascontiguousarray(b2.reshape(100, 1)),
        "b3c": np.ascontiguousarray(b3.reshape(1, 1)),
        "idn": np.eye(nbags, dtype=np.float32),
    }
    in_maps = []
    for cid in range(ncores):
        shard = feats[cid * nbags : (cid + 1) * nbags].reshape(nbags * ntiles, fsz)
        q = shard.astype(ml_dtypes.float8_e4m3)
        # ft8[h*16+c, p, n_in] = q[h*SLEN + n_in, c*128 + p]
        a = q.reshape(nbags, SLEN, NCHUNK, 128).transpose(0, 2, 3, 1)
        a = np.ascontiguousarray(a).reshape(NSTRIPE, 128, SLEN)
        in_maps.append({**base, "ft8": a})
    return in_maps


def _run(inputs, trace=False, **spmd_kwargs):
    from concourse.bass_utils import run_bass_kernel_spmd

    nc = _build_nc(BAGS_PER_CORE, NTILES, FSZ)
    in_maps = _make_in_maps(inputs, BAGS_PER_CORE, NTILES, FSZ, NCORES)
    res = run_bass_kernel_spmd(
        nc, in_maps, list(range(NCORES)), trace=trace, **spmd_kwargs
    )
    logits = np.concatenate(
        [res.results[c]["logits"].reshape(BAGS_PER_CORE, 1) for c in range(NCORES)],
        axis=0,
    )
    probs = np.concatenate(
        [res.results[c]["probs"].reshape(BAGS_PER_CORE, 1) for c in range(NCORES)],
        axis=0,
    )
    return (logits, probs), res


def kernel(**inputs):
    out, _ = _run(inputs, trace=False)
    return out


# revision 14
# speedup vs baseline: 1.3401x; 1.2200x over previous
# Trainium2 Bass kernel for nn_Model_26190710571339 (topk_masking).
#
# Model: scores = einsum('bnf,f->bn', feats, w_conv); per-bag sort -> bottom-5
# and top-5 score values -> tiny MLP (10->200->100->1, sigmoid) -> logits, probs.
#
# Sharding: data-parallel over the bag axis; 2 bags per NeuronCore x 8 cores.
# Weights replicated.
#
# v5: the stationary-feats design (one 128-column LDWEIGHTS + 1-column
# matmul per 128 tiles = 8192 PE instructions) was limited by PE
# INSTRUCTION FETCH: the sequencer streams 512 KB of instructions from HBM
# at ~16 KB per ~6.6 us, stalling the PE ~2.8 us per stripe. This version
# flips the matmul orientation so one instruction covers 512 tiles x 256
# features:
#   - feats staged as fp8_e4m3 in DoubleRow layout: rhs [128p=f, 2 k-tiles,
#     512 n] per matmul, lhsT = w pair [128, 2] fp8 (x64, rescaled in the
#     PSUM->SBUF copy). fp8 DoubleRow runs the PE at 2x bf16 rate.
#     512 matmuls + 512 ldweights total -> ~70 KB instruction stream.
#   - scores land in PSUM rows [1, 512] (partition 0), accumulated over the
#     8 chunk-pairs within one 2048-tile window; windows stream as 2 MB
#     cp-quad sub-DMAs with 16 KB contiguous lines on the 2 HWDGE queues.
#   - top/bottom-5 via DVE max8 on the window score rows (top-8 of each
#     2048-tile window, then top-8 of the window candidates per bag —
#     exact containment). Bottom side via a negated copy (ACT engine,
#     Copy activation with scale=-1/64). The last window runs max8
#     per 512-row to shorten the serial tail.
#   - the 10x2 MLP input is assembled directly by two tiny DMAs per bag
#     (no PE transpose); descending order + sign are folded into a host
#     permutation of W1's columns. Quantization rel err ~7e-3 (gate 2e-2).

import numpy as np

B = 16
NTILES = 16384
FSZ = 2048
R = 5
NCORES = 8
BAGS_PER_CORE = B // NCORES  # 2

NWIN = 16            # windows per core
WINN = 2048          # tiles per window
NSUB = 2             # cp-quad sub-DMAs per window
NCP = 8              # chunk pairs (256 features each)
ROWS = WINN // 512   # psum rows per window (4)
WSCALE = 64.0        # w is staged as fp8(64*w); copies rescale by 1/64


def _build_nc(nbags, ntiles, fsz, bufs=8, ncores=NCORES):
    import concourse.mybir as mybir
    import concourse.tile as tile
    from concourse import bacc
    from contextlib import ExitStack

    f32 = mybir.dt.float32
    f8 = mybir.dt.float8e4
    Act = mybir.ActivationFunctionType
    DR = mybir.MatmulPerfMode.DoubleRow

    win_per_bag = NWIN // nbags  # 8

    nc = bacc.Bacc("TRN2", target_bir_lowering=False, debug=False, num_devices=ncores)
    # ft8[w, k, p, (c4, j, n)]: feats8[w*WINN + n, ((k*4+c4)*2+j)*128 + p]
    ft8 = nc.declare_dram_parameter("ft8", [NWIN, NSUB, 128, 4 * 2 * WINN], f8, isOutput=False)
    w8 = nc.declare_dram_parameter("w8", [128, 2, 16], f8, isOutput=False)
    w1t = nc.declare_dram_parameter("w1t", [2 * R, 200], f32, isOutput=False)
    w2ta = nc.declare_dram_parameter("w2ta", [128, 100], f32, isOutput=False)
    w2tb = nc.declare_dram_parameter("w2tb", [72, 100], f32, isOutput=False)
    w3t = nc.declare_dram_parameter("w3t", [100, 1], f32, isOutput=False)
    b1a = nc.declare_dram_parameter("b1a", [128, 1], f32, isOutput=False)
    b1b = nc.declare_dram_parameter("b1b", [72, 1], f32, isOutput=False)
    b2c = nc.declare_dram_parameter("b2c", [100, 1], f32, isOutput=False)
    b3c = nc.declare_dram_parameter("b3c", [1, 1], f32, isOutput=False)
    logits_o = nc.declare_dram_parameter("logits", [1, nbags], f32, isOutput=True)
    probs_o = nc.declare_dram_parameter("probs", [1, nbags], f32, isOutput=True)

    with ExitStack() as ctx:
        tc = ctx.enter_context(tile.TileContext(nc))
        consts = ctx.enter_context(tc.tile_pool(name="consts", bufs=1))

        w8_sb = consts.tile([128, 2, 16], f8)
        nc.sync.dma_start(w8_sb[:], w8[:])

        fpool = ctx.enter_context(tc.tile_pool(name="fpool", bufs=bufs))
        wpool = ctx.enter_context(tc.tile_pool(name="wpool", bufs=2))
        psmain = ctx.enter_context(tc.tile_pool(name="psmain", bufs=4, space="PSUM"))
        tpool = ctx.enter_context(tc.tile_pool(name="tpool", bufs=1))
        hwdge = [nc.sync, nc.scalar]

        # per-bag window-candidate rows: 7 windows x 8 + last window 4 rows x 8
        wt8max = [tpool.tile([1, 96], f32, name=f"wt8max{b}") for b in range(nbags)]
        wt8min = [tpool.tile([1, 96], f32, name=f"wt8min{b}") for b in range(nbags)]

        ndma = 0
        for w in range(NWIN):
            bag = w // win_per_bag
            last_of_bag = (w + 1) % win_per_bag == 0
            subs = []
            for k in range(NSUB):
                sub = fpool.tile([128, 4, 2, WINN], f8, name="sub")
                hwdge[ndma % 2].dma_start(sub[:], ft8[w, k])
                ndma += 1
                subs.append(sub)
            winrow = wpool.tile([1, WINN], f32, name="winrow")
            negwin = wpool.tile([1, WINN], f32, name="negwin")
            pr = [psmain.tile([128, 512], f32, name="pr") for _ in range(ROWS)]
            for k in range(NSUB):
                for s in range(ROWS):
                    for c4 in range(4):
                        cp = k * 4 + c4
                        nc.tensor.matmul(
                            pr[s][0:1, :],
                            lhsT=w8_sb[:, :, cp],
                            rhs=subs[k][:, c4, :, s * 512 : (s + 1) * 512],
                            start=(cp == 0),
                            stop=(cp == NCP - 1),
                            perf_mode=DR,
                        )
                    if k == NSUB - 1:
                        rsl = slice(s * 512, (s + 1) * 512)
                        nc.vector.tensor_scalar_mul(
                            winrow[0:1, rsl], pr[s][0:1, :], 1.0 / WSCALE
                        )
                        nc.scalar.activation(
                            negwin[0:1, rsl], pr[s][0:1, :], Act.Copy,
                            scale=-1.0 / WSCALE,
                        )
                        if w == NWIN - 1:
                            # last window: per-row candidates to shorten the
                            # serial tail after the final sub-DMA lands
                            o = 56 + s * 8
                            nc.vector.max(wt8max[bag][0:1, o : o + 8], winrow[0:1, rsl])
                            nc.vector.max(wt8min[bag][0:1, o : o + 8], negwin[0:1, rsl])
            if w != NWIN - 1:
                o = (w % win_per_bag) * 8
                nc.vector.max(wt8max[bag][0:1, o : o + 8], winrow[:])
                nc.vector.max(wt8min[bag][0:1, o : o + 8], negwin[:])

        # MLP consts on the idle SWDGE queue
        w1t_sb = consts.tile([2 * R, 200], f32)
        nc.gpsimd.dma_start(w1t_sb[:], w1t[:])
        w2ta_sb = consts.tile([128, 100], f32)
        nc.gpsimd.dma_start(w2ta_sb[:], w2ta[:])
        w2tb_sb = consts.tile([72, 100], f32)
        nc.gpsimd.dma_start(w2tb_sb[:], w2tb[:])
        w3t_sb = consts.tile([100, 1], f32)
        nc.gpsimd.dma_start(w3t_sb[:], w3t[:])
        b1a_sb = consts.tile([128, 1], f32)
        nc.gpsimd.dma_start(b1a_sb[:], b1a[:])
        b1b_sb = consts.tile([72, 1], f32)
        nc.gpsimd.dma_start(b1b_sb[:], b1b[:])
        b2c_sb = consts.tile([100, 1], f32)
        nc.gpsimd.dma_start(b2c_sb[:], b2c[:])
        b3c_sb = consts.tile([1, 1], f32)
        nc.gpsimd.dma_start(b3c_sb[:], b3c[:])

        # ---- global top/bottom-8 per bag over the window candidates
        fmax = [tpool.tile([1, 8], f32, name=f"fmax{b}") for b in range(nbags)]
        fmin = [tpool.tile([1, 8], f32, name=f"fmin{b}") for b in range(nbags)]
        for b in range(nbags):
            ncand = 56 + ROWS * 8 if b == nbags - 1 else 64
            nc.vector.max(fmax[b][:], wt8max[b][0:1, 0:ncand])
            nc.vector.max(fmin[b][:], wt8min[b][0:1, 0:ncand])

        # mmT[j, b] = -(j+1)-th smallest for j<5; (10-j)-th largest for j>=5
        # (host-permuted W1 compensates sign and order)
        mmT = tpool.tile([2 * R, nbags], f32)
        for b in range(nbags):
            hwdge[b % 2].dma_start(mmT[0:R, b : b + 1], fmin[b][0:1, 0:R])
            hwdge[b % 2].dma_start(mmT[R : 2 * R, b : b + 1], fmax[b][0:1, 0:R])

        # ---- MLP (transposed): hT = sigmoid(W @ xT + b), biases per-partition
        psum2 = ctx.enter_context(tc.tile_pool(name="psum2", bufs=1, space="PSUM"))
        h1pa = psum2.tile([128, nbags], f32, name="h1pa")
        h1pb = psum2.tile([72, nbags], f32, name="h1pb")
        nc.tensor.matmul(h1pa[:], lhsT=w1t_sb[:, 0:128], rhs=mmT[:], start=True, stop=True)
        nc.tensor.matmul(h1pb[:], lhsT=w1t_sb[:, 128:200], rhs=mmT[:], start=True, stop=True)
        h1a = tpool.tile([128, nbags], f32)
        h1b = tpool.tile([72, nbags], f32)
        nc.scalar.activation(h1a[:], h1pa[:], Act.Sigmoid, bias=b1a_sb[:], scale=1.0)
        nc.scalar.activation(h1b[:], h1pb[:], Act.Sigmoid, bias=b1b_sb[:], scale=1.0)

        h2p = psum2.tile([100, nbags], f32, name="h2p")
        nc.tensor.matmul(h2p[:], lhsT=w2ta_sb[:], rhs=h1a[:], start=True, stop=False)
        nc.tensor.matmul(h2p[:], lhsT=w2tb_sb[:], rhs=h1b[:], start=False, stop=True)
        h2 = tpool.tile([100, nbags], f32)
        nc.scalar.activation(h2[:], h2p[:], Act.Sigmoid, bias=b2c_sb[:], scale=1.0)

        lp = psum2.tile([1, nbags], f32, name="lp")
        nc.tensor.matmul(lp[:], lhsT=w3t_sb[:], rhs=h2[:], start=True, stop=True)
        lsb = tpool.tile([1, nbags], f32)
        nc.vector.tensor_scalar_add(lsb[:], lp[:], b3c_sb[:])
        psb = tpool.tile([1, nbags], f32)
        nc.scalar.activation(psb[:], lsb[:], Act.Sigmoid)

        nc.sync.dma_start(logits_o[:], lsb[:])
        nc.scalar.dma_start(probs_o[:], psb[:])

    nc.finalize()
    return nc


def _make_in_maps(inputs, nbags, ntiles, fsz, ncores):
    import ml_dtypes

    feats = np.asarray(inputs["feats"], dtype=np.float32)
    w_conv = np.asarray(inputs["w_conv"], dtype=np.float32)
    W1 = np.asarray(inputs["W1"], dtype=np.float32)
    b1 = np.asarray(inputs["b1"], dtype=np.float32)
    W2 = np.asarray(inputs["W2"], dtype=np.float32)
    b2 = np.asarray(inputs["b2"], dtype=np.float32)
    W3 = np.asarray(inputs["W3"], dtype=np.float32)
    b3 = np.asarray(inputs["b3"], dtype=np.float32)

    # Kernel produces mmT[j, b] = -(bottom-(j+1)) for j<5 and
    # top-(j-4)-largest (descending) for j>=5; reference minmax is bottom-5
    # ascending then top-5 ascending. Fold into W1's columns.
    W1_hw = np.empty_like(W1)
    W1_hw[:, 0:R] = -W1[:, 0:R]
    W1_hw[:, R : 2 * R] = W1[:, 2 * R - 1 : R - 1 : -1]

    # w8[p, j, cp] = fp8(WSCALE * w[cp*256 + j*128 + p]), cp slots padded to
    # 16 so the DoubleRow ldweights k-tile step is 16 elements (ISA rule)
    w8v = (WSCALE * w_conv).reshape(NCP, 2, 128).transpose(2, 1, 0)
    w8 = np.zeros((128, 2, 16), dtype=np.float32)
    w8[:, :, :NCP] = w8v
    w8 = np.ascontiguousarray(w8).astype(ml_dtypes.float8_e4m3)

    base = {
        "w8": w8,
        "w1t": np.ascontiguousarray(W1_hw.T),
        "w2ta": np.ascontiguousarray(W2.T[:128]),
        "w2tb": np.ascontiguousarray(W2.T[128:]),
        "w3t": np.ascontiguousarray(W3.T),
        "b1a": np.ascontiguousarray(b1[:128].reshape(128, 1)),
        "b1b": np.ascontiguousarray(b1[128:].reshape(72, 1)),
        "b2c": np.ascontiguousarray(b2.reshape(100, 1)),
        "b3c": np.ascontiguousarray(b3.reshape(1, 1)),
    }
    in_maps = []
    for cid in range(ncores):
        shard = feats[cid * nbags : (cid + 1) * nbags].reshape(nbags * ntiles, fsz)
        q = shard.astype(ml_dtypes.float8_e4m3)
        # [w, n, k, c4, j, p] -> [w, k, p, c4, j, n]
        a = q.reshape(NWIN, WINN, NSUB, 4, 2, 128).transpose(0, 2, 5, 3, 4, 1)
        a = np.ascontiguousarray(a).reshape(NWIN, NSUB, 128, 4 * 2 * WINN)
        in_maps.append({**base, "ft8": a})
    return in_maps


def _run(inputs, trace=False, **spmd_kwargs):
    from concourse.bass_utils import run_bass_kernel_spmd

    nc = _build_nc(BAGS_PER_CORE, NTILES, FSZ)
    in_maps = _make_in_maps(inputs, BAGS_PER_CORE, NTILES, FSZ, NCORES)
    res = run_bass_kernel_spmd(
        nc, in_maps, list(range(NCORES)), trace=trace, **spmd_kwargs
    )
    logits = np.concatenate(
        [res.results[c]["logits"].reshape(BAGS_PER_CORE, 1) for c in range(NCORES)],
        axis=0,
    )
    probs = np.concatenate(
        [res.results[c]["probs"].reshape(BAGS_PER_CORE, 1) for c in range(NCORES)],
        axis=0,
    )
    return (logits, probs), res


def kernel(**inputs):
    out, _ = _run(inputs, trace=False)
    return out
